# revision 26
# baseline (speedup 1.0000x reference)
"""Block-sparse attention kernel for TRN2 (8 NeuronCores, SPMD).

Math (from the reference nn.Module):
  x [1, 8, 512, 768] -> flatten to [S=4096, 768]
  q/k/v = x @ W{q,k,v}.T, split into H=12 heads of D=64
  block mask: query in view v attends keys [0 : 512*P_v] where
  P_v = 2 for v in {0,1}, v+1 for v >= 2  (always a prefix of key blocks)
  out = softmax(q k^T / 8 + mask) v, merge heads, @ Wo.T + bo

Sharding: core c owns query chunk c (64 queries) of EVERY view, i.e. rows
v*512 + c*64 .. +64 for v in 0..8 (512 queries/core). Per-core work is
then exactly balanced (sum_v P_v = 37 key blocks of attention each) and —
because each view's allowed prefix is static — no mask input is needed at
all: for key block kb only views v with P_v > kb participate, which is a
contiguous query-column range.

Every core computes the full K/V projections (replicated; avoids
collectives). Dataflow is fully transposed (x^T, K^T, Q^T, out^T) so every
projection matmul has its contraction dim on partitions and a >=384-wide
moving operand (float32r at full PE rate). Q/K/V are rounded to bf16 in
the PSUM->SBUF copy; scores and attn*V run in bf16 (full PE rate at any
moving width, FWL-eligible on HW), accumulating in f32 PSUM.

Softmax sums come for free from a ones column interleaved into the V
tiles (head h occupies columns h*65..+64 of each 128-key sub-block, column
h*65+64 is 1.0), so the AV matmul's 65th output row is the per-query sum
of exp. attn*V accumulates over the 4 key sub-blocks in PSUM (start/stop
flags); per key block one vector add folds it into the SBUF accumulator.

Each view is normalized (1/sum broadcast via a ones outer product,
applied on the vector engine) right after its last key block, so the
whole normalize hides under later blocks' attention; the output
projection runs in two halves (views 0-3 during key block 5, views 4-7
in the epilogue). Startup DMAs are spread across the gpsimd/SP/Act
queues. CoreSim: ~251 us/core, PE 94% busy (PE floor for this
algorithm ~236 us; collectives to deshard the replicated K/V would cost
more than the 107 us of PE they save, per the TRN2 collective cost
model).
"""

import sys

sys.path.insert(0, "/opt/trn_rl_repo")

import numpy as np

import concourse.bass as bass
import concourse.mybir as mybir
import concourse.tile as tile
from concourse.bass_utils import run_bass_kernel_spmd

F32 = mybir.dt.float32
F32R = mybir.dt.float32r
BF16 = mybir.dt.bfloat16

S, DIM, H, D = 4096, 768, 12, 64
V, L = 8, 512
NC_N = 8
NM = DIM // 128          # 6 chunks of the model dim
NKB = S // 512           # 8 key blocks
SCALE = float(D) ** -0.5
CH = 64                  # queries per (view, core) chunk

# allowed 512-key prefix blocks per view
PV = [2, 2, 3, 4, 5, 6, 7, 8]
# first active view per key block: {v : PV[v] > kb} = [AKB[kb], 8)
AKB = [0, 0, 2, 3, 4, 5, 6, 7]


def legalize_multiwaits(nc):
    """This toolchain's walrus accepts at most ONE sync-wait per
    instruction; Tile's sem-assignment happily emits several. Split the
    extras into standalone EventSemaphore (wait) instructions on the same
    engine, placed immediately before the gated instruction."""
    scratch = nc.alloc_semaphore("legalize_scratch")
    fn = nc.m.functions[0]
    for bb in fn.blocks:
        insts = list(bb.instructions)
        out = []
        changed = False
        for inst in insts:
            si = getattr(inst, "sync_info", None)
            ow = list(si.on_wait) if si is not None and si.on_wait else []
            if len(ow) > 1:
                for w in ow[:-1]:
                    ev = nc.engines[inst.engine].nop(nofuse=True)
                    raw = ev.ins
                    raw.sync_info = mybir.SyncInfo(on_wait=[w], on_update=[])
                    # pop it from wherever the builder appended it
                    tail = nc.cur_bb.bb.instructions
                    assert tail[-1].name == raw.name
                    nc.cur_bb.bb.instructions = tail[:-1]
                    out.append(raw)
                si.on_wait = [ow[-1]]
                inst.sync_info = si
                changed = True
            out.append(inst)
        if changed:
            bb.instructions = out


def build_program(nkb=NKB, loop_n=1):
    nc = bass.Bass()
    xT = nc.dram_tensor("xT", [DIM, S], BF16, kind="ExternalInput")
    xTq = nc.dram_tensor("xTq", [DIM, L], BF16, kind="ExternalInput")
    WqT = nc.dram_tensor("WqT", [DIM, DIM], BF16, kind="ExternalInput")
    WkT = nc.dram_tensor("WkT", [DIM, DIM], BF16, kind="ExternalInput")
    WvT = nc.dram_tensor("WvT", [DIM, DIM], BF16, kind="ExternalInput")
    WoT = nc.dram_tensor("WoT", [DIM, DIM], F32, kind="ExternalInput")
    boT = nc.dram_tensor("boT", [128, NM], F32, kind="ExternalInput")
    outT = nc.dram_tensor("outT", [DIM, L], F32, kind="ExternalOutput")

    def mm(out, lhsT, rhs, start, stop):
        nc.tensor.matmul(out, lhsT, rhs, start=start, stop=stop)

    with nc.allow_low_precision(reason="bf16 q/k/v/attn (checked: 4e-3 rel)"), \
         tile.TileContext(nc) as tc, \
         tc.tile_pool(name="const", bufs=1) as cpool, \
         tc.tile_pool(name="wres", bufs=1) as wres, \
         tc.tile_pool(name="wstream", bufs=6) as wstr, \
         tc.tile_pool(name="acc", bufs=1) as accp, \
         tc.tile_pool(name="xt", bufs=2) as xtp, \
         tc.tile_pool(name="ktblk", bufs=2) as ktp, \
         tc.tile_pool(name="vblk", bufs=2) as vbp, \
         tc.tile_pool(name="expp", bufs=2) as expp, \
         tc.tile_pool(name="outp", bufs=2) as outp, \
         tc.tile_pool(name="ps_proj", bufs=2, space="PSUM") as psproj, \
         tc.tile_pool(name="ps_sc", bufs=2, space="PSUM") as pssc, \
         tc.tile_pool(name="ps_av", bufs=2, space="PSUM") as psav:

        ones_r = cpool.tile([1, 64], BF16, name="ones_r")
        nc.vector.memset(ones_r[:, :], 1.0)
        bo_sb = cpool.tile([128, NM], F32, name="bo_sb")
        nc.sync.dma_start(bo_sb[:, :], boT[:, :])

        # resident K/V weights: block cc at cols cc*DIM, rows = W*T rows.
        # Spread the startup loads across several engine DMA queues so the
        # transfers overlap (a single queue serializes the prologue).
        wk_sb = wres.tile([128, NM * DIM], BF16, name="wk_sb")
        wv_sb = wres.tile([128, NM * DIM], BF16, name="wv_sb")
        for cc in range(NM):
            nc.scalar.dma_start(
                wk_sb[:, cc * DIM:(cc + 1) * DIM], WkT[cc * 128:(cc + 1) * 128, :]
            )
            nc.scalar.dma_start(
                wv_sb[:, cc * DIM:(cc + 1) * DIM], WvT[cc * 128:(cc + 1) * 128, :]
            )

        for _rep in range(loop_n):
            # ---- Q projection: Q^T[mi block] = sum_cc WqT[cc,mi].T @ xTq[cc] ----
            # qt cols [mi*L + v*CH : +CH] = view v's 64 queries, m-chunk mi
            qt_sb = accp.tile([128, NM * L], BF16, name="qt_sb")
            xq_sb = xtp.tile([128, NM * L], BF16, name="xt_t", tag="xt")
            for cc in range(NM):
                nc.sync.dma_start(
                    xq_sb[:, cc * L:(cc + 1) * L], xTq[cc * 128:(cc + 1) * 128, :]
                )
            wq_t = []
            for cc in range(NM):
                w = wstr.tile([128, DIM], BF16, name=f"wq_{cc}", tag="wstr")
                nc.gpsimd.dma_start(w[:, :], WqT[cc * 128:(cc + 1) * 128, :])
                wq_t.append(w)
            for mi in range(NM):
                psq = psproj.tile([128, L], F32, name="psq", tag="proj")
                for cc in range(NM):
                    mm(psq[:, :], wq_t[cc][:, mi * 128:(mi + 1) * 128],
                       xq_sb[:, cc * L:(cc + 1) * L], cc == 0, cc == NM - 1)
                nc.vector.tensor_copy(qt_sb[:, mi * L:(mi + 1) * L], psq[:, :])

            # stream Wo early; gpsimd so the f32 -> f32r load rounds (walrus
            # rejects plain-f32 producers feeding f32r matmuls). Used by the
            # split output projection at kb==5 and in the epilogue.
            wo_t = []
            for cc in range(NM):
                w = wstr.tile([128, DIM], F32R, name=f"wo_{cc}", tag="wstr")
                nc.gpsimd.dma_start(w[:, :], WoT[cc * 128:(cc + 1) * 128, :])
                wo_t.append(w)

            def emit_out_proj(c0, width):
                # outT[:, c0:c0+width] = Wo @ at_acc[:, c0:c0+width] + bo;
                # only valid once the views covering those columns are
                # normalized
                for mmi in range(NM):
                    pso = psproj.tile([128, width], F32, name="pso", tag="proj")
                    for cc in range(NM):
                        mm(pso[:, :],
                           wo_t[cc][:, mmi * 128:(mmi + 1) * 128],
                           at_acc[:, cc * L + c0: cc * L + c0 + width],
                           cc == 0, cc == NM - 1)
                    out_t = outp.tile([128, width], F32, name="out_t", tag="out")
                    nc.vector.tensor_scalar_add(out_t[:, :], pso[:, :],
                                                bo_sb[:, mmi:mmi + 1])
                    nc.sync.dma_start(
                        outT[mmi * 128:(mmi + 1) * 128, c0:c0 + width],
                        out_t[:, :])

            # persistent accumulators
            # at_acc cols [mi*L + v*CH : +CH]: m-chunk mi, view v's queries
            at_acc = accp.tile([128, NM * L], F32R, name="at_acc")
            # softmax sums: head h on partition (h%2)*64, cols (h//2)*L + q
            sums_sb = accp.tile([65, NM * L], F32, name="sums_sb")
            nc.vector.memset(sums_sb[:, :], 1.0)
            recip_sb = accp.tile([65, NM * L], BF16, name="recip_sb")

            # ---- key-block loop ----
            for kb in range(nkb):
                a = AKB[kb]          # active views = [a, 8)
                w = (V - a) * CH     # active query columns

                xt_b = xtp.tile([128, NM * L], BF16, name="xt_t", tag="xt")
                for cc in range(NM):
                    nc.gpsimd.dma_start(
                        xt_b[:, cc * L:(cc + 1) * L],
                        xT[cc * 128:(cc + 1) * 128, kb * 512:(kb + 1) * 512],
                    )

                # K^T block: [dims(part, by mi), 512 keys], bf16
                kt_b = ktp.tile([128, NM * 512], BF16, name="kt_b", tag="kt")
                for mi in range(NM):
                    psk = psproj.tile([128, 512], F32, name="psk", tag="proj")
                    for cc in range(NM):
                        mm(psk[:, :],
                           wk_sb[:, cc * DIM + mi * 128: cc * DIM + (mi + 1) * 128],
                           xt_b[:, cc * L:(cc + 1) * L], cc == 0, cc == NM - 1)
                    nc.vector.tensor_copy(kt_b[:, mi * 512:(mi + 1) * 512], psk[:, :])

                # V block (bf16): 4 sub-chunks of 128 keys; head h at cols
                # sc*H*65 + h*65 ..+64, col +64 is 1.0 (softmax-sum trick)
                v_b = vbp.tile([128, 4 * H * (D + 1)], BF16, name="v_b", tag="v")
                ones_cols = v_b[:, :].rearrange(
                    "p (s h j) -> p s h j", s=4, j=D + 1)[:, :, :, D:D + 1]
                nc.vector.memset(ones_cols, 1.0)
                v_sc = [v_b[:, sc * H * (D + 1):(sc + 1) * H * (D + 1)]
                        for sc in range(4)]
                for sc in range(4):
                    v_t = v_sc[sc]
                    for half in range(2):
                        psv = psproj.tile([128, 512], F32, name="psv", tag="proj")
                        for cc in range(NM):
                            mm(psv[:, 0:384],
                               xt_b[:, cc * L + sc * 128: cc * L + (sc + 1) * 128],
                               wv_sb[:, cc * DIM + half * 384: cc * DIM + (half + 1) * 384],
                               cc == 0, cc == NM - 1)
                        dst = v_t[:, half * 6 * (D + 1):(half + 1) * 6 * (D + 1)]
                        dst = dst.rearrange("p (h j) -> p h j", j=D + 1)[:, :, 0:D]
                        srcp = psv[:, 0:384].rearrange("p (h j) -> p h j", j=D)
                        nc.vector.tensor_copy(dst, srcp)

                # attention for all heads against this key block, active
                # views only (columns a*CH..512 of the query dim)
                for h in range(H):
                    po = (h % 2) * 64            # partition offset of head h
                    co = (h // 2) * 512          # kt col offset (m-chunk h//2)
                    sp = (h % 2) * 64
                    qcol = (h // 2) * L + a * CH
                    ps_o = psav.tile([65, 512], F32, name="ps_o", tag="av")
                    for scp in range(2):
                        ps_s = pssc.tile([128, 1024], F32, name="ps_s", tag="sc")
                        for s2 in range(2):
                            sc = scp * 2 + s2
                            mm(ps_s[:, s2 * 512: s2 * 512 + w],
                               kt_b[po:po + 64, co + sc * 128: co + (sc + 1) * 128],
                               qt_sb[po:po + 64, qcol:qcol + w], True, True)
                        exp_t = expp.tile([128, 2 * w], BF16, name="exp_t", tag="exp")
                        nc.scalar.activation(
                            exp_t[:, :].rearrange("p (s q) -> p s q", s=2),
                            ps_s[:, :].rearrange("p (s q) -> p s q", s=2)[:, :, 0:w],
                            mybir.ActivationFunctionType.Exp,
                            scale=SCALE,
                        )
                        for s2 in range(2):
                            sc = scp * 2 + s2
                            mm(ps_o[:, 0:w], v_sc[sc][:, h * 65:(h + 1) * 65],
                               exp_t[:, s2 * w:(s2 + 1) * w],
                               scp == 0 and s2 == 0, scp == 1 and s2 == 1)
                    acol = (h // 2) * L + a * CH
                    if kb == 0:
                        nc.vector.tensor_copy(at_acc[po:po + 64, acol:acol + w],
                                              ps_o[0:64, 0:w])
                        nc.vector.tensor_copy(
                            sums_sb[sp:sp + 1, acol:acol + w],
                            ps_o[64:65, 0:w])
                    else:
                        nc.vector.tensor_add(at_acc[po:po + 64, acol:acol + w],
                                             at_acc[po:po + 64, acol:acol + w],
                                             ps_o[0:64, 0:w])
                        nc.vector.tensor_add(
                            sums_sb[sp:sp + 1, acol:acol + w],
                            sums_sb[sp:sp + 1, acol:acol + w],
                            ps_o[64:65, 0:w])

                # normalize views whose key prefix ends at this block
                # (P_v == kb+1): their at_acc/sums columns are final, so the
                # whole normalize hides under later blocks' attention.
                for v in [vv for vv in range(V) if PV[vv] == kb + 1]:
                    sl = (lambda t: t.rearrange("p (m q) -> p m q", q=L)
                          [:, :, v * CH:(v + 1) * CH])
                    nc.vector.reciprocal(sl(recip_sb[:, :]), sl(sums_sb[:, :]))
                    # broadcast 1/sum across partitions via outer product
                    # with ones; head h -> rb cols (h%2)*512 + (h//2)*64
                    rb_ps = pssc.tile([128, 1024], F32, name="rb_ps", tag="sc")
                    for h in range(H):
                        sp = (h % 2) * 64
                        stage_r = outp.tile([1, CH], BF16, name="stage_r",
                                            tag="stg")
                        nc.vector.tensor_copy(
                            stage_r[0:1, :],
                            recip_sb[sp:sp + 1,
                                     (h // 2) * L + v * CH:
                                     (h // 2) * L + (v + 1) * CH])
                        mm(rb_ps[0:64,
                                 (h % 2) * 512 + (h // 2) * 64:
                                 (h % 2) * 512 + (h // 2) * 64 + 64],
                           ones_r[0:1, :], stage_r[0:1, :], True, True)
                    for sub in range(2):
                        rb_v = rb_ps[0:64, sub * 512: sub * 512 + 384]
                        rb_v = rb_v.rearrange("p (m q) -> p m q", q=CH)
                        at_v = sl(at_acc[sub * 64:(sub + 1) * 64, :])
                        nc.vector.tensor_mul(at_v, at_v, rb_v)

                # views 0-3 (at_acc columns 0..256 of each m-chunk) are all
                # normalized once kb==3 is done: project+store them during
                # block 5 so half the epilogue hides under blocks 6-7
                if kb == 5:
                    emit_out_proj(0, 4 * CH)

            # ---- output projection for the remaining views 4-7 ----
            emit_out_proj(4 * CH, 4 * CH)

    legalize_multiwaits(nc)
    return nc


_program = None


def make_in_maps(x, Wq, Wk, Wv, Wo, bo):
    import ml_dtypes

    bf16 = ml_dtypes.bfloat16
    xf = np.ascontiguousarray(np.asarray(x, np.float32).reshape(S, DIM))
    xT = np.ascontiguousarray(xf.T.astype(bf16))
    shared = {
        "xT": xT,
        "WqT": np.ascontiguousarray(np.asarray(Wq, np.float32).T.astype(bf16)),
        "WkT": np.ascontiguousarray(np.asarray(Wk, np.float32).T.astype(bf16)),
        "WvT": np.ascontiguousarray(np.asarray(Wv, np.float32).T.astype(bf16)),
        "WoT": np.ascontiguousarray(np.asarray(Wo, np.float32).T),
        "boT": np.ascontiguousarray(
            np.asarray(bo, np.float32).reshape(NM, 128).T
        ),
    }
    in_maps = []
    for c in range(NC_N):
        m = dict(shared)
        m["xTq"] = np.ascontiguousarray(np.concatenate(
            [xT[:, v * L + c * CH: v * L + (c + 1) * CH] for v in range(V)],
            axis=1))
        in_maps.append(m)
    return in_maps


def kernel(x, Wq, Wk, Wv, Wo, bo):
    global _program
    in_maps = make_in_maps(x, Wq, Wk, Wv, Wo, bo)
    if _program is None:
        _program = build_program()
    ret = run_bass_kernel_spmd(_program, in_maps, list(range(NC_N)))
    out = np.empty((S, DIM), np.float32)
    for c in range(NC_N):
        oT = ret.results[c]["outT"]
        for v in range(V):
            out[v * L + c * CH: v * L + (c + 1) * CH, :] = (
                oT[:, v * CH:(v + 1) * CH].T)
    return out.reshape(1, V, L, DIM)


# revision 28
# speedup vs baseline: 1.0932x; 1.0932x over previous
"""Block-sparse attention kernel for TRN2 (8 NeuronCores, SPMD).

Math (from the reference nn.Module):
  x [1, 8, 512, 768] -> flatten to [S=4096, 768]
  q/k/v = x @ W{q,k,v}.T, split into H=12 heads of D=64
  block mask: query in view v attends keys [0 : 512*P_v] where
  P_v = 2 for v in {0,1}, v+1 for v >= 2  (always a prefix of key blocks)
  out = softmax(q k^T / 8 + mask) v, merge heads, @ Wo.T + bo

Sharding: core c owns query chunk c (64 queries) of EVERY view, i.e. rows
v*512 + c*64 .. +64 for v in 0..8 (512 queries/core). Per-core work is
then exactly balanced (sum_v P_v = 37 key blocks of attention each) and —
because each view's allowed prefix is static — no mask input is needed at
all: for key block kb only views v with P_v > kb participate, which is a
contiguous query-column range.

Every core computes the full K/V projections (replicated; avoids
collectives). Dataflow is fully transposed (x^T, K^T, Q^T, out^T) so every
projection matmul has its contraction dim on partitions and a >=384-wide
moving operand (float32r at full PE rate). Q/K/V are rounded to bf16 in
the PSUM->SBUF copy; scores and attn*V run in bf16 (full PE rate at any
moving width, FWL-eligible on HW), accumulating in f32 PSUM.

Softmax sums come for free from a ones column interleaved into the V
tiles (head h occupies columns h*65..+64 of each 128-key sub-block, column
h*65+64 is 1.0), so the AV matmul's 65th output row is the per-query sum
of exp. attn*V accumulates over the 4 key sub-blocks in PSUM (start/stop
flags); per key block one vector add folds it into the SBUF accumulator.

Each view is normalized (1/sum broadcast via a ones outer product,
applied on the vector engine) right after its last key block, so the
whole normalize hides under later blocks' attention; the output
projection runs in two halves (views 0-3 during key block 5, views 4-7
in the epilogue). Startup DMAs are spread across the gpsimd/SP/Act
queues. CoreSim: ~251 us/core, PE 94% busy (PE floor for this
algorithm ~236 us; collectives to deshard the replicated K/V would cost
more than the 107 us of PE they save, per the TRN2 collective cost
model).
"""

import sys

sys.path.insert(0, "/opt/trn_rl_repo")

import numpy as np

import concourse.bass as bass
import concourse.mybir as mybir
import concourse.tile as tile
from concourse.bass_utils import run_bass_kernel_spmd

F32 = mybir.dt.float32
F32R = mybir.dt.float32r
BF16 = mybir.dt.bfloat16

S, DIM, H, D = 4096, 768, 12, 64
V, L = 8, 512
NC_N = 8
NM = DIM // 128          # 6 chunks of the model dim
NKB = S // 512           # 8 key blocks
SCALE = float(D) ** -0.5
CH = 64                  # queries per (view, core) chunk

# allowed 512-key prefix blocks per view
PV = [2, 2, 3, 4, 5, 6, 7, 8]
# first active view per key block: {v : PV[v] > kb} = [AKB[kb], 8)
AKB = [0, 0, 2, 3, 4, 5, 6, 7]


def legalize_multiwaits(nc):
    """This toolchain's walrus accepts at most ONE sync-wait per
    instruction; Tile's sem-assignment happily emits several. Split the
    extras into standalone EventSemaphore (wait) instructions on the same
    engine, placed immediately before the gated instruction."""
    scratch = nc.alloc_semaphore("legalize_scratch")
    fn = nc.m.functions[0]
    for bb in fn.blocks:
        insts = list(bb.instructions)
        out = []
        changed = False
        for inst in insts:
            si = getattr(inst, "sync_info", None)
            ow = list(si.on_wait) if si is not None and si.on_wait else []
            if len(ow) > 1:
                for w in ow[:-1]:
                    ev = nc.engines[inst.engine].nop(nofuse=True)
                    raw = ev.ins
                    raw.sync_info = mybir.SyncInfo(on_wait=[w], on_update=[])
                    # pop it from wherever the builder appended it
                    tail = nc.cur_bb.bb.instructions
                    assert tail[-1].name == raw.name
                    nc.cur_bb.bb.instructions = tail[:-1]
                    out.append(raw)
                si.on_wait = [ow[-1]]
                inst.sync_info = si
                changed = True
            out.append(inst)
        if changed:
            bb.instructions = out


def build_program(nkb=NKB, loop_n=1):
    nc = bass.Bass()
    xT = nc.dram_tensor("xT", [DIM, S], BF16, kind="ExternalInput")
    xTq = nc.dram_tensor("xTq", [DIM, L], BF16, kind="ExternalInput")
    WqT = nc.dram_tensor("WqT", [DIM, DIM], BF16, kind="ExternalInput")
    WkT = nc.dram_tensor("WkT", [DIM, DIM], BF16, kind="ExternalInput")
    WvT = nc.dram_tensor("WvT", [DIM, DIM], BF16, kind="ExternalInput")
    WoT = nc.dram_tensor("WoT", [DIM, DIM], F32, kind="ExternalInput")
    boT = nc.dram_tensor("boT", [128, NM], F32, kind="ExternalInput")
    outT = nc.dram_tensor("outT", [DIM, L], F32, kind="ExternalOutput")

    def mm(out, lhsT, rhs, start, stop):
        nc.tensor.matmul(out, lhsT, rhs, start=start, stop=stop)

    with nc.allow_low_precision(reason="bf16 q/k/v/attn (checked: 4e-3 rel)"), \
         tile.TileContext(nc) as tc, \
         tc.tile_pool(name="const", bufs=1) as cpool, \
         tc.tile_pool(name="wres", bufs=1) as wres, \
         tc.tile_pool(name="wstream", bufs=6) as wstr, \
         tc.tile_pool(name="acc", bufs=1) as accp, \
         tc.tile_pool(name="xt", bufs=2) as xtp, \
         tc.tile_pool(name="ktblk", bufs=2) as ktp, \
         tc.tile_pool(name="vblk", bufs=2) as vbp, \
         tc.tile_pool(name="expp", bufs=2) as expp, \
         tc.tile_pool(name="outp", bufs=2) as outp, \
         tc.tile_pool(name="ps_proj", bufs=2, space="PSUM") as psproj, \
         tc.tile_pool(name="ps_sc", bufs=2, space="PSUM") as pssc, \
         tc.tile_pool(name="ps_av", bufs=2, space="PSUM") as psav:

        ones_r = cpool.tile([1, 64], BF16, name="ones_r")
        nc.vector.memset(ones_r[:, :], 1.0)
        bo_sb = cpool.tile([128, NM], F32, name="bo_sb")
        nc.sync.dma_start(bo_sb[:, :], boT[:, :])

        # resident K/V weights: block cc at cols cc*DIM, rows = W*T rows.
        # Spread the startup loads across several engine DMA queues so the
        # transfers overlap (a single queue serializes the prologue).
        wk_sb = wres.tile([128, NM * DIM], BF16, name="wk_sb")
        wv_sb = wres.tile([128, NM * DIM], BF16, name="wv_sb")
        for cc in range(NM):
            nc.scalar.dma_start(
                wk_sb[:, cc * DIM:(cc + 1) * DIM], WkT[cc * 128:(cc + 1) * 128, :]
            )
            nc.scalar.dma_start(
                wv_sb[:, cc * DIM:(cc + 1) * DIM], WvT[cc * 128:(cc + 1) * 128, :]
            )

        for _rep in range(loop_n):
            # ---- Q projection: Q^T[mi block] = sum_cc WqT[cc,mi].T @ xTq[cc] ----
            # qt cols [mi*L + v*CH : +CH] = view v's 64 queries, m-chunk mi
            qt_sb = accp.tile([128, NM * L], BF16, name="qt_sb")
            xq_sb = xtp.tile([128, NM * L], BF16, name="xt_t", tag="xt")
            for cc in range(NM):
                nc.sync.dma_start(
                    xq_sb[:, cc * L:(cc + 1) * L], xTq[cc * 128:(cc + 1) * 128, :]
                )
            wq_t = []
            for cc in range(NM):
                w = wstr.tile([128, DIM], BF16, name=f"wq_{cc}", tag="wstr")
                nc.gpsimd.dma_start(w[:, :], WqT[cc * 128:(cc + 1) * 128, :])
                wq_t.append(w)
            for mi in range(NM):
                psq = psproj.tile([128, L], F32, name="psq", tag="proj")
                for cc in range(NM):
                    mm(psq[:, :], wq_t[cc][:, mi * 128:(mi + 1) * 128],
                       xq_sb[:, cc * L:(cc + 1) * L], cc == 0, cc == NM - 1)
                nc.vector.tensor_copy(qt_sb[:, mi * L:(mi + 1) * L], psq[:, :])

            # stream Wo early; gpsimd so the f32 -> f32r load rounds (walrus
            # rejects plain-f32 producers feeding f32r matmuls). Used by the
            # split output projection at kb==5 and in the epilogue.
            wo_t = []
            for cc in range(NM):
                w = wstr.tile([128, DIM], F32R, name=f"wo_{cc}", tag="wstr")
                nc.gpsimd.dma_start(w[:, :], WoT[cc * 128:(cc + 1) * 128, :])
                wo_t.append(w)

            def emit_out_proj(c0, width):
                # outT[:, c0:c0+width] = Wo @ at_acc[:, c0:c0+width] + bo;
                # only valid once the views covering those columns are
                # normalized
                for mmi in range(NM):
                    pso = psproj.tile([128, width], F32, name="pso", tag="proj")
                    for cc in range(NM):
                        mm(pso[:, :],
                           wo_t[cc][:, mmi * 128:(mmi + 1) * 128],
                           at_acc[:, cc * L + c0: cc * L + c0 + width],
                           cc == 0, cc == NM - 1)
                    out_t = outp.tile([128, width], F32, name="out_t", tag="out")
                    nc.vector.tensor_scalar_add(out_t[:, :], pso[:, :],
                                                bo_sb[:, mmi:mmi + 1])
                    nc.sync.dma_start(
                        outT[mmi * 128:(mmi + 1) * 128, c0:c0 + width],
                        out_t[:, :])

            # persistent accumulators
            # at_acc cols [mi*L + v*CH : +CH]: m-chunk mi, view v's queries
            at_acc = accp.tile([128, NM * L], F32R, name="at_acc")
            # softmax sums: head h on partition (h%2)*64, cols (h//2)*L + q
            sums_sb = accp.tile([65, NM * L], F32, name="sums_sb")
            nc.vector.memset(sums_sb[:, :], 1.0)
            recip_sb = accp.tile([65, NM * L], BF16, name="recip_sb")

            # ---- key-block loop ----
            for kb in range(nkb):
                a = AKB[kb]          # active views = [a, 8)
                w = (V - a) * CH     # active query columns

                xt_b = xtp.tile([128, NM * L], BF16, name="xt_t", tag="xt")
                for cc in range(NM):
                    nc.gpsimd.dma_start(
                        xt_b[:, cc * L:(cc + 1) * L],
                        xT[cc * 128:(cc + 1) * 128, kb * 512:(kb + 1) * 512],
                    )

                # K^T block: [dims(part, by mi), 512 keys], bf16
                kt_b = ktp.tile([128, NM * 512], BF16, name="kt_b", tag="kt")
                for mi in range(NM):
                    psk = psproj.tile([128, 512], F32, name="psk", tag="proj")
                    for cc in range(NM):
                        mm(psk[:, :],
                           wk_sb[:, cc * DIM + mi * 128: cc * DIM + (mi + 1) * 128],
                           xt_b[:, cc * L:(cc + 1) * L], cc == 0, cc == NM - 1)
                    nc.vector.tensor_copy(kt_b[:, mi * 512:(mi + 1) * 512], psk[:, :])

                # V block (bf16): 4 sub-chunks of 128 keys; head h at cols
                # sc*H*65 + h*65 ..+64, col +64 is 1.0 (softmax-sum trick)
                v_b = vbp.tile([128, 4 * H * (D + 1)], BF16, name="v_b", tag="v")
                ones_cols = v_b[:, :].rearrange(
                    "p (s h j) -> p s h j", s=4, j=D + 1)[:, :, :, D:D + 1]
                nc.vector.memset(ones_cols, 1.0)
                v_sc = [v_b[:, sc * H * (D + 1):(sc + 1) * H * (D + 1)]
                        for sc in range(4)]
                for sc in range(4):
                    v_t = v_sc[sc]
                    for half in range(2):
                        psv = psproj.tile([128, 512], F32, name="psv", tag="proj")
                        for cc in range(NM):
                            mm(psv[:, 0:384],
                               xt_b[:, cc * L + sc * 128: cc * L + (sc + 1) * 128],
                               wv_sb[:, cc * DIM + half * 384: cc * DIM + (half + 1) * 384],
                               cc == 0, cc == NM - 1)
                        dst = v_t[:, half * 6 * (D + 1):(half + 1) * 6 * (D + 1)]
                        dst = dst.rearrange("p (h j) -> p h j", j=D + 1)[:, :, 0:D]
                        srcp = psv[:, 0:384].rearrange("p (h j) -> p h j", j=D)
                        nc.vector.tensor_copy(dst, srcp)

                # attention for all heads against this key block, active
                # views only (columns a*CH..512 of the query dim)
                for h in range(H):
                    po = (h % 2) * 64            # partition offset of head h
                    co = (h // 2) * 512          # kt col offset (m-chunk h//2)
                    sp = (h % 2) * 64
                    qcol = (h // 2) * L + a * CH
                    ps_o = psav.tile([65, 512], F32, name="ps_o", tag="av")
                    for scp in range(2):
                        ps_s = pssc.tile([128, 1024], F32, name="ps_s", tag="sc")
                        for s2 in range(2):
                            sc = scp * 2 + s2
                            mm(ps_s[:, s2 * 512: s2 * 512 + w],
                               kt_b[po:po + 64, co + sc * 128: co + (sc + 1) * 128],
                               qt_sb[po:po + 64, qcol:qcol + w], True, True)
                        exp_t = expp.tile([128, 2 * w], BF16, name="exp_t", tag="exp")
                        nc.scalar.activation(
                            exp_t[:, :].rearrange("p (s q) -> p s q", s=2),
                            ps_s[:, :].rearrange("p (s q) -> p s q", s=2)[:, :, 0:w],
                            mybir.ActivationFunctionType.Exp,
                            scale=SCALE,
                        )
                        for s2 in range(2):
                            sc = scp * 2 + s2
                            mm(ps_o[:, 0:w], v_sc[sc][:, h * 65:(h + 1) * 65],
                               exp_t[:, s2 * w:(s2 + 1) * w],
                               scp == 0 and s2 == 0, scp == 1 and s2 == 1)
                    acol = (h // 2) * L + a * CH
                    if kb == 0:
                        nc.vector.tensor_copy(at_acc[po:po + 64, acol:acol + w],
                                              ps_o[0:64, 0:w])
                        nc.vector.tensor_copy(
                            sums_sb[sp:sp + 1, acol:acol + w],
                            ps_o[64:65, 0:w])
                    else:
                        nc.vector.tensor_add(at_acc[po:po + 64, acol:acol + w],
                                             at_acc[po:po + 64, acol:acol + w],
                                             ps_o[0:64, 0:w])
                        nc.vector.tensor_add(
                            sums_sb[sp:sp + 1, acol:acol + w],
                            sums_sb[sp:sp + 1, acol:acol + w],
                            ps_o[64:65, 0:w])

                # normalize views whose key prefix ends at this block
                # (P_v == kb+1): their at_acc/sums columns are final, so the
                # whole normalize hides under later blocks' attention.
                for v in [vv for vv in range(V) if PV[vv] == kb + 1]:
                    sl = (lambda t: t.rearrange("p (m q) -> p m q", q=L)
                          [:, :, v * CH:(v + 1) * CH])
                    nc.vector.reciprocal(sl(recip_sb[:, :]), sl(sums_sb[:, :]))
                    # broadcast 1/sum across partitions via outer product
                    # with ones; head h -> rb cols (h%2)*512 + (h//2)*64
                    rb_ps = pssc.tile([128, 1024], F32, name="rb_ps", tag="sc")
                    for h in range(H):
                        sp = (h % 2) * 64
                        stage_r = outp.tile([1, CH], BF16, name="stage_r",
                                            tag="stg")
                        nc.vector.tensor_copy(
                            stage_r[0:1, :],
                            recip_sb[sp:sp + 1,
                                     (h // 2) * L + v * CH:
                                     (h // 2) * L + (v + 1) * CH])
                        mm(rb_ps[0:64,
                                 (h % 2) * 512 + (h // 2) * 64:
                                 (h % 2) * 512 + (h // 2) * 64 + 64],
                           ones_r[0:1, :], stage_r[0:1, :], True, True)
                    for sub in range(2):
                        rb_v = rb_ps[0:64, sub * 512: sub * 512 + 384]
                        rb_v = rb_v.rearrange("p (m q) -> p m q", q=CH)
                        at_v = sl(at_acc[sub * 64:(sub + 1) * 64, :])
                        nc.vector.tensor_mul(at_v, at_v, rb_v)

                # views 0-3 (at_acc columns 0..256 of each m-chunk) are all
                # normalized once kb==3 is done: project+store them during
                # block 5 so half the epilogue hides under blocks 6-7
                if kb == 5:
                    emit_out_proj(0, 4 * CH)

            # ---- output projection for the remaining views 4-7 ----
            emit_out_proj(4 * CH, 4 * CH)

    legalize_multiwaits(nc)
    return nc


_program = None


def make_in_maps(x, Wq, Wk, Wv, Wo, bo):
    import ml_dtypes

    bf16 = ml_dtypes.bfloat16
    xf = np.ascontiguousarray(np.asarray(x, np.float32).reshape(S, DIM))
    xT = np.ascontiguousarray(xf.T.astype(bf16))
    shared = {
        "xT": xT,
        "WqT": np.ascontiguousarray(np.asarray(Wq, np.float32).T.astype(bf16)),
        "WkT": np.ascontiguousarray(np.asarray(Wk, np.float32).T.astype(bf16)),
        "WvT": np.ascontiguousarray(np.asarray(Wv, np.float32).T.astype(bf16)),
        "WoT": np.ascontiguousarray(np.asarray(Wo, np.float32).T),
        "boT": np.ascontiguousarray(
            np.asarray(bo, np.float32).reshape(NM, 128).T
        ),
    }
    in_maps = []
    for c in range(NC_N):
        m = dict(shared)
        m["xTq"] = np.ascontiguousarray(np.concatenate(
            [xT[:, v * L + c * CH: v * L + (c + 1) * CH] for v in range(V)],
            axis=1))
        in_maps.append(m)
    return in_maps


def kernel(x, Wq, Wk, Wv, Wo, bo):
    global _program
    in_maps = make_in_maps(x, Wq, Wk, Wv, Wo, bo)
    if _program is None:
        _program = build_program()
    ret = run_bass_kernel_spmd(_program, in_maps, list(range(NC_N)))
    out = np.empty((S, DIM), np.float32)
    for c in range(NC_N):
        oT = ret.results[c]["outT"]
        for v in range(V):
            out[v * L + c * CH: v * L + (c + 1) * CH, :] = (
                oT[:, v * CH:(v + 1) * CH].T)
    return out.reshape(1, V, L, DIM)
